# revision 11
# baseline (speedup 1.0000x reference)
"""Trainium2 Bass kernel for DiceLoss (hard-argmax dice, ignore background, mean).

Problem (hardcoded shapes):
  y_true: [16, 512, 512] int32 in [0, 8)
  y_pred: [16, 8, 512, 512] float32
  out   : scalar float32 = mean over classes 1..7 of
          (2*tp + eps) / (2*tp + fp + fn + eps)
  with pred_cls = argmax_c y_pred, one-hot tp/fp/fn sums over all pixels.

Strategy (8 NeuronCores, data-parallel over batch):
  - Each core processes 2 of the 16 batch images (SPMD, same NEFF), streamed
    as 5 chunks of [512, 1024, 1024, 1024, 512] pixel-columns (small head
    chunk to start compute early, small tail chunk to shorten the
    end-of-stream compute that cannot overlap DMA; the middle chunk spans
    the image boundary -- pixels are pixels for global tp/fp/fn sums).
  - y_pred is cast f32 -> fp16 during the DMA itself (SWDGE CME cast, one
    8-channel DMA per chunk; HBM read traffic unchanged).  fp16
    equality-vs-max introduces spurious argmax ties at ~5e-4 of pixels ->
    rel err ~5e-4 on the final dice (tolerance 2e-2).  Labels are staged
    as uint8 (lossless re-encoding of values 0..7) and cast uint8 -> fp16
    in-DMA.
  - DVE (VectorE), all ops in measured fast perf modes (accum_out is never
    used: it drops DVE to 1x mode on HW; scalar_tensor_tensor is 1x-only,
    so masks use tensor_tensor/tensor_scalar):
      * 7-op pairwise tensor_tensor MAX tree (fp16, 2x)
      * pred masks (ch[c] == max): batched tensor_tensor IS_EQUAL (fp16, 2x)
      * gt masks (labels == c): 7 tensor_scalar IS_EQUAL (fp16, 4x)
  - Mask layout: per class, 9 subtiles of 128 columns = [127 px | 1 ones
    col]; a 1024-px chunk = 8 full subtiles + an 8-px tail, a 512-px chunk
    = 4 full + 4-px tail (pad columns kept zero).  Pred tile P and gt tile
    G use the same layout.
  - TensorE: per (class, subtile) one matmul psum_c += P_cs^T @ G_cs
    (N=128) accumulated over all subtiles/chunks.  In the [128,128] psum:
    diag[0:127] = tp, col 127 = per-col pred counts, row 127 = per-col gt
    counts -- all three statistics from the same matmul stream.
  - Host: sums the 8 cores' exact-integer partials and forms the dice mean
    in float32, mirroring the reference arithmetic.
"""

import numpy as np

EPS = 1e-05

# Problem geometry (hardcoded per the harness contract).
N_CORES = 8
NB = 2             # batch images per core
C = 8              # classes
P = 128            # SBUF partitions
FP = 2048          # free-dim elements per image plane ([128, 2048] = 512*512)
FMAX = 1024        # max pixels per chunk (class stride inside chall)
SUBMAX = 9         # subtiles per class block: 8 x 127 px + tail
BW = SUBMAX * 128  # mask-tile columns per class block (1152)

# Chunk schedule: list of (pieces, px) where pieces = [(img, off, len), ...].
# 512-px head/tail chunks shorten the non-overlappable pipeline ends; the
# middle chunk spans the image boundary.
CHUNKS = [
    ([(0, 0, 512)], 512),
    ([(0, 512, 1024)], 1024),
    ([(0, 1536, 512), (1, 0, 512)], 1024),
    ([(1, 512, 1024)], 1024),
    ([(1, 1536, 512)], 512),
]

BATCH_PRED = True  # single 4D-AP pred compare vs 7 per-class ops

_CACHED_NC = None


def build_bass():
    """Build the Bass kernel (same NEFF for all 8 cores)."""
    from contextlib import ExitStack

    import concourse.bacc as bacc
    import concourse.tile as tile
    from concourse import mybir

    nc = bacc.Bacc(None, target_bir_lowering=False)

    yp = nc.dram_tensor("yp", [NB, C, P, FP], mybir.dt.float32, kind="ExternalInput")
    yt = nc.dram_tensor("yt", [NB, P, FP], mybir.dt.uint8, kind="ExternalInput")
    # per class: [128, 128] psum (diag = tp, col 127 = pred cnt, row 127 = gt cnt)
    tp_out = nc.dram_tensor("tp_out", [7, P, 128], mybir.dt.float32, kind="ExternalOutput")

    with tile.TileContext(nc) as tc, ExitStack() as ctx:
        chpool = ctx.enter_context(tc.tile_pool(name="ch", bufs=3))
        tpool = ctx.enter_context(tc.tile_pool(name="tt", bufs=3))
        mpool = ctx.enter_context(tc.tile_pool(name="mx", bufs=2))
        mtmp = ctx.enter_context(tc.tile_pool(name="mtmp", bufs=2))
        maskp = ctx.enter_context(tc.tile_pool(name="mask", bufs=1))
        drainp = ctx.enter_context(tc.tile_pool(name="drain", bufs=1))
        psump = ctx.enter_context(tc.tile_pool(name="psum", bufs=1, space="PSUM"))

        # Persistent mask tiles (single buffer; ones/zero cols survive reuse).
        Pm = maskp.tile([P, 7 * BW], mybir.dt.float16, name="Pm", tag="Pm")
        Gm = maskp.tile([P, 7 * BW], mybir.dt.float16, name="Gm", tag="Gm")
        # cvec[:, i] = i+1 (class constants for the batched gt tail compare)
        cvec = maskp.tile([P, 7], mybir.dt.float16, name="cvec", tag="cvec")

        # One-time init (cheap, column-targeted: a full-tile memset costs
        # ~7us of DVE and would delay the first chunk's compute):
        #   - ones column (col 127 of each subtile block)
        #   - zero pads of the two tail subtiles (s=4 cols[4:], s=8 cols[8:])
        # Mask ops only ever write px columns, so pads stay 0 / ones stay 1.
        for t in (Pm, Gm):
            blocks = t[:, :].rearrange("p (c s w) -> p c s w", c=7, w=128)
            nc.vector.memset(blocks[:, :, :, 127:128], 1.0)
            nc.vector.memset(blocks[:, :, 4, 4:127], 0.0)
            nc.vector.memset(blocks[:, :, 8, 8:127], 0.0)
        for i in range(7):
            nc.vector.memset(cvec[:, i : i + 1], float(i + 1))

        # PSUM is bank-granular (8 banks x 2KB/partition): pack 4 classes of
        # [128,128] f32 (512B/part) per bank.
        psA = psump.tile([P, 4 * 128], mybir.dt.float32, name="psA", tag="psA")
        psB = psump.tile([P, 3 * 128], mybir.dt.float32, name="psB", tag="psB")
        psums = [psA[:, i * 128 : (i + 1) * 128] for i in range(4)] + [
            psB[:, i * 128 : (i + 1) * 128] for i in range(3)
        ]

        nchunks = len(CHUNKS)
        for k, (pieces, fk) in enumerate(CHUNKS):
            n_full = fk // 127          # full 127-px subtiles
            tail = fk - 127 * n_full    # tail pixels (4 or 8)
            main = 127 * n_full
            nsub = n_full + 1           # matmul subtiles incl. tail

            # ---- loads: one 8-channel cast DMA per piece (f32 -> fp16),
            # classes laid at stride FMAX inside chall ----
            chall = chpool.tile([P, C * FMAX], mybir.dt.float16, name="chall", tag="chall")
            tf = tpool.tile([P, FMAX], mybir.dt.float16, name="tf", tag="tf")
            dst = 0
            for (n, off, ln) in pieces:
                ch_dst = chall[:, :].rearrange("p (c w) -> p c w", c=C)[
                    :, :, dst : dst + ln
                ]
                nc.gpsimd.dma_start(
                    out=ch_dst,
                    in_=yp[n][:, :, off : off + ln].rearrange("c p w -> p c w"),
                )
                nc.gpsimd.dma_start(
                    out=tf[:, dst : dst + ln], in_=yt[n][:, off : off + ln]
                )
                dst += ln

            ch = [chall[:, c * FMAX : c * FMAX + fk] for c in range(C)]

            # ---- max tree (DVE, fp16 tensor_tensor => 2x mode) ----
            m01 = mtmp.tile([P, FMAX], mybir.dt.float16, name="m01", tag="m01")
            nc.vector.tensor_max(m01[:, 0:fk], ch[0], ch[1])
            m23 = mtmp.tile([P, FMAX], mybir.dt.float16, name="m23", tag="m23")
            nc.vector.tensor_max(m23[:, 0:fk], ch[2], ch[3])
            m45 = mtmp.tile([P, FMAX], mybir.dt.float16, name="m45", tag="m45")
            nc.vector.tensor_max(m45[:, 0:fk], ch[4], ch[5])
            m67 = mtmp.tile([P, FMAX], mybir.dt.float16, name="m67", tag="m67")
            nc.vector.tensor_max(m67[:, 0:fk], ch[6], ch[7])
            m0123 = mtmp.tile([P, FMAX], mybir.dt.float16, name="m0123", tag="m0123")
            nc.vector.tensor_max(m0123[:, 0:fk], m01[:, 0:fk], m23[:, 0:fk])
            m4567 = mtmp.tile([P, FMAX], mybir.dt.float16, name="m4567", tag="m4567")
            nc.vector.tensor_max(m4567[:, 0:fk], m45[:, 0:fk], m67[:, 0:fk])
            m = mpool.tile([P, FMAX], mybir.dt.float16, name="m", tag="m")
            nc.vector.tensor_max(m[:, 0:fk], m0123[:, 0:fk], m4567[:, 0:fk])

            # ---- stale-pad zeroing: a short chunk after a longer one leaves
            # old px values in its tail subtile's pad cols ----
            if k > 0 and fk < CHUNKS[k - 1][1]:
                for t in (Pm, Gm):
                    blocks = t[:, :].rearrange("p (c s w) -> p c s w", c=7, w=128)
                    nc.vector.memset(blocks[:, :, n_full, tail:127], 0.0)

            p_blocks = Pm[:, :].rearrange("p (c s w) -> p c s w", c=7, w=128)
            g_blocks = Gm[:, :].rearrange("p (c s w) -> p c s w", c=7, w=128)

            # ---- batched tail compares (all 7 classes, 2 ops) ----
            p_tails = p_blocks[:, :, n_full, 0:tail]
            ch_tails = chall[:, :].rearrange("p (c w) -> p c w", c=C)[
                :, 1:C, main : main + tail
            ]
            m_tail_b = m[:, main:fk].unsqueeze(1).broadcast_to([P, 7, tail])
            nc.vector.tensor_tensor(
                p_tails, ch_tails, m_tail_b, op=mybir.AluOpType.is_equal
            )
            g_tails = g_blocks[:, :, n_full, 0:tail]
            tf_tail_b = tf[:, main:fk].unsqueeze(1).broadcast_to([P, 7, tail])
            cvec_b = cvec[:, :].unsqueeze(2).broadcast_to([P, 7, tail])
            nc.vector.tensor_tensor(
                g_tails, tf_tail_b, cvec_b, op=mybir.AluOpType.is_equal
            )

            # ---- main mask compares ----
            if BATCH_PRED:
                # all 7 classes in one 4D-AP tensor_tensor
                p_main = Pm[:, :].rearrange(
                    "p (c s w) -> p c s w", c=7, w=128
                )[:, :, 0:n_full, 0:127]
                ch_main = chall[:, FMAX : C * FMAX].rearrange(
                    "p (c w) -> p c w", c=7
                )[:, :, 0:main].rearrange("p c (s w) -> p c s w", w=127)
                m_main_b = (
                    m[:, 0:main]
                    .rearrange("p (s w) -> p s w", w=127)
                    .unsqueeze(1)
                    .broadcast_to([P, 7, n_full, 127])
                )
                nc.vector.tensor_tensor(
                    p_main, ch_main, m_main_b, op=mybir.AluOpType.is_equal
                )
            else:
                m_main = m[:, 0:main].rearrange("p (s w) -> p s w", w=127)
                for c in range(1, C):
                    nc.vector.tensor_tensor(
                        p_blocks[:, c - 1, 0:n_full, 0:127],
                        ch[c][:, 0:main].rearrange("p (s w) -> p s w", w=127),
                        m_main,
                        op=mybir.AluOpType.is_equal,
                    )

            # gt masks: 7 tensor_scalar (4x mode)
            tf_main = tf[:, 0:main].rearrange("p (s w) -> p s w", w=127)
            for c in range(1, C):
                nc.vector.tensor_scalar(
                    out=g_blocks[:, c - 1, 0:n_full, 0:127],
                    in0=tf_main,
                    scalar1=float(c),
                    scalar2=0.0,
                    op0=mybir.AluOpType.is_equal,
                    op1=mybir.AluOpType.add,
                )

            # ---- PE: one N=128 matmul per (class, subtile) ----
            for c in range(1, C):
                blk = (c - 1) * BW
                for s in range(nsub):
                    nc.tensor.matmul(
                        psums[c - 1],
                        lhsT=Pm[:, blk + s * 128 : blk + (s + 1) * 128],
                        rhs=Gm[:, blk + s * 128 : blk + (s + 1) * 128],
                        start=(k == 0 and s == 0),
                        stop=(k == nchunks - 1 and s == nsub - 1),
                    )

        for c in range(7):
            tps = drainp.tile([P, 128], mybir.dt.float32, name=f"tps{c}", tag=f"tps{c}")
            nc.scalar.copy(out=tps, in_=psums[c])
            nc.sync.dma_start(out=tp_out[c], in_=tps)

    nc.finalize()
    return nc


def _get_bass():
    global _CACHED_NC
    if _CACHED_NC is None:
        _CACHED_NC = build_bass()
    return _CACHED_NC


def make_in_maps(y_true, y_pred):
    yp = np.ascontiguousarray(np.asarray(y_pred, dtype=np.float32))
    # labels are 0..7: uint8 re-encoding is lossless
    yt = np.asarray(y_true).astype(np.uint8)
    in_maps = []
    for i in range(N_CORES):
        yps = np.ascontiguousarray(yp[NB * i : NB * (i + 1)]).reshape(NB, C, P, FP)
        yts = np.ascontiguousarray(yt[NB * i : NB * (i + 1)]).reshape(NB, P, FP)
        in_maps.append({"yp": yps, "yt": yts})
    return in_maps


def epilogue(results):
    """Combine the 8 cores' partial sums into the final dice mean (float32,
    mirroring the reference arithmetic)."""
    tp = np.zeros(7, dtype=np.float64)
    pred_cnt = np.zeros(7, dtype=np.float64)
    gt_cnt = np.zeros(7, dtype=np.float64)
    for r in results:
        po = np.asarray(r["tp_out"], dtype=np.float64)  # [7, 128, 128]
        tp += np.trace(po[:, 0:127, 0:127], axis1=1, axis2=2)
        pred_cnt += po[:, 0:127, 127].sum(axis=1)
        gt_cnt += po[:, 127, 0:127].sum(axis=1)

    tp32 = tp.astype(np.float32)
    fp32_ = (pred_cnt - tp).astype(np.float32)
    fn32 = (gt_cnt - tp).astype(np.float32)
    eps = np.float32(EPS)
    two = np.float32(2.0)
    dice = (two * tp32 + eps) / (two * tp32 + fp32_ + fn32 + eps)
    return np.asarray(np.mean(dice, dtype=np.float32), dtype=np.float32)


def kernel(**inputs):
    from concourse.bass_utils import run_bass_kernel_spmd

    nc = _get_bass()
    in_maps = make_in_maps(inputs["y_true"], inputs["y_pred"])
    res = run_bass_kernel_spmd(nc, in_maps, core_ids=list(range(N_CORES)))
    return epilogue(res.results)


if __name__ == "__main__":
    # smoke test with random data
    rng = np.random.default_rng(0)
    y_true = rng.integers(0, C, size=(16, 512, 512)).astype(np.int32)
    y_pred = rng.standard_normal((16, C, 512, 512)).astype(np.float32)
    out = kernel(y_true=y_true, y_pred=y_pred)
    print("kernel output:", out)
